# revision 12
# baseline (speedup 1.0000x reference)
"""Trainium2 Bass kernel for DeepHedgingModel (LSTM scan, B=8192 T=512 F=4 H=32).

Strategy (pure data parallel over 8 cores, 1024 batch rows per core):

Per core, the 1024-row batch is split into 4 "bands" of 256 columns; band j
owns SBUF/PSUM partitions [32j, 32j+32).  All per-gate matmuls are M=32
tile_position matmuls whose outputs land inside the band, so every
elementwise LSTM op is partition-aligned and runs over all 4 bands in a
single [128, 256] instruction.

Gate-type column order in G is (g, i, f, o) so one Tanh covers cols 0:256
and one Sigmoid covers cols 256:1024.

z-vector per band = rows [d, x0..x3, ones] at partitions 32j..32j+6.
BatchNorm (inference affine) is folded into the stationary weights; the
d-feedback sigmoid is computed as tanh: sigma(y) = 0.5 + 0.5*tanh(y/2),
and the 0.5/0.5 affine is folded into the d-row weights + bias row, so the
raw tanh output is stored as the recurrent "d" row.  The output path
spreads d over partitions with a one-hot e_tau x d rank-1 matmul
accumulated in PSUM over 32 steps, then a DVE 32x32 stream-transpose moves
batch onto partitions; the 0.5+0.5x affine is applied once at the end.
"""

import os
import sys
from contextlib import ExitStack

import numpy as np

sys.path.insert(0, "/opt/trn_rl_repo")

import concourse.bass as bass  # noqa: E402
import concourse.tile as tile  # noqa: E402
from concourse import bacc, mybir  # noqa: E402

F32 = mybir.dt.float32
AF = mybir.ActivationFunctionType
ALU = mybir.AluOpType

EPS = 1e-5


# ----------------------------------------------------------------------------
# Config
# ----------------------------------------------------------------------------
class Cfg:
    def __init__(self, ncol=256, T=512, sblk=16, tau=32, nbands=4):
        self.ncol = ncol          # batch columns per band
        self.T = T                # timesteps
        self.sblk = sblk          # steps per x-DMA block
        self.tau = tau            # steps per d-spread epoch (<= 32)
        self.nbands = nbands      # 4 bands of 32 partitions
        self.B = nbands * ncol    # per-core batch
        assert T % sblk == 0 and T % tau == 0


FULL = Cfg()


# ----------------------------------------------------------------------------
# Host-side weight folding / input prep
# ----------------------------------------------------------------------------
def fold_params(p):
    """Return dict of numpy arrays for the SBUF-resident constants."""
    H = 32
    # gate-type order in G columns: (g, i, f, o); torch rows are (i, f, g, o)
    perm = np.concatenate([
        np.arange(2 * H, 3 * H),   # g
        np.arange(0, H),           # i
        np.arange(H, 2 * H),       # f
        np.arange(3 * H, 4 * H),   # o
    ])
    W_ih = p["W_ih"].astype(np.float64)
    W_hh = p["W_hh"].astype(np.float64)
    b_ih = p["b_ih"].astype(np.float64)
    b_hh = p["b_hh"].astype(np.float64)
    gam = p["bn_gamma"].astype(np.float64)
    bet = p["bn_beta"].astype(np.float64)
    mu = p["bn_mean"].astype(np.float64)
    var = p["bn_var"].astype(np.float64)
    a = gam / np.sqrt(var + EPS)          # [5]
    b_a = bet - mu * a                    # [5]

    Wx_eff = W_ih[:, :4] * a[None, :4]    # [128, 4]
    w_d_eff = W_ih[:, 4] * a[4]           # [128]
    C = b_a @ W_ih.T + b_ih + b_hh        # [128]
    # recurrence stores t_y = tanh(y/2);  d = 0.5 + 0.5 t_y
    d_row = 0.5 * w_d_eff                 # weight on stored t_y
    C = C + 0.5 * w_d_eff                 # constant part of d contribution

    # scale the tanh-gate (g) rows?  no: direct tanh table is used.
    out = {}
    # Whh_sb [128, 128]: [32j+k, 32g+m] = W_hh[perm[32g+m], k]
    Whh_band = W_hh[perm, :].T            # [k 32, 128 gates(permuted)]
    out["whh"] = np.tile(Whh_band[:, :], (4, 1)).astype(np.float32)  # [128,128]
    # Wx1d_sb [128, 128]: band rows r: 0=d_row, 1..4=x, 5=ones(C)
    wx1d_band = np.zeros((32, 128))
    wx1d_band[0, :] = d_row[perm]
    wx1d_band[1:5, :] = Wx_eff[perm, :].T
    wx1d_band[5, :] = C[perm]
    out["wx1d"] = np.tile(wx1d_band, (4, 1)).astype(np.float32)      # [128,128]
    # Wm1_sb [128, 32]: [32j+k, m] = W1[m, k]
    W1 = p["W1"].astype(np.float64)
    out["wm1"] = np.tile(W1.T, (4, 1)).astype(np.float32)            # [128,32]
    # Wm2_sb [128, 1]: [32j+k, 0] = W2[0, k]
    W2 = p["W2"].astype(np.float64)
    out["wm2"] = np.tile(W2.T, (4, 1)).astype(np.float32)            # [128,1]
    # per-partition b1 bias column [128,1]
    out["b1col"] = np.tile(p["b1"].astype(np.float32)[:, None], (4, 1))
    out["b2half"] = float(0.5 * p["b2"].astype(np.float64)[0])
    return out


def make_eye(cfg):
    """E_sb [128, tau*32]: row 32j holds one-hot e_tau blocks."""
    E = np.zeros((128, cfg.tau * 32), np.float32)
    for t in range(cfg.tau):
        E[::32, t * 32 + t] = 1.0
    return E


def prep_x(x_core, cfg):
    """x_core [B, T, 4] -> xprep [T/sblk, nbands, 5, sblk, ncol] (ones in row 4)."""
    B, T, F = x_core.shape
    nb, nc_, sb = cfg.nbands, cfg.ncol, cfg.sblk
    xp = np.empty((T // sb, nb, 5, sb, nc_), np.float32)
    # x_core[b, t, f] with b = j*ncol + n
    xr = x_core.reshape(nb, nc_, T // sb, sb, F)
    xp[:, :, :4] = xr.transpose(2, 0, 4, 3, 1)  # [blk, band, f, s, n]
    xp[:, :, 4] = 1.0
    return xp


# ----------------------------------------------------------------------------
# Kernel body
# ----------------------------------------------------------------------------
def build_kernel(nc, cfg, use_strided=False):
    """Declare DRAM I/O and emit the TileContext program."""
    N = cfg.ncol
    T, SB, TAU = cfg.T, cfg.sblk, cfg.tau

    d_x = nc.dram_tensor("xprep", [T // SB, 4, 5, SB, N], F32, kind="ExternalInput")
    d_whh = nc.dram_tensor("whh", [128, 128], F32, kind="ExternalInput")
    d_wx1d = nc.dram_tensor("wx1d", [128, 128], F32, kind="ExternalInput")
    d_wm1 = nc.dram_tensor("wm1", [128, 32], F32, kind="ExternalInput")
    d_wm2 = nc.dram_tensor("wm2", [128, 1], F32, kind="ExternalInput")
    d_b1 = nc.dram_tensor("b1col", [128, 1], F32, kind="ExternalInput")
    d_eye = nc.dram_tensor("eye", [128, TAU * 32], F32, kind="ExternalInput")
    d_b2h = nc.dram_tensor("b2half", [128, 1], F32, kind="ExternalInput")
    d_out = nc.dram_tensor("dout", [cfg.B, T], F32, kind="ExternalOutput")

    with tile.TileContext(nc) as tc, ExitStack() as ctx:
        wp = ctx.enter_context(tc.tile_pool(name="weights", bufs=1))
        zp = ctx.enter_context(tc.tile_pool(name="zb", bufs=3))
        sp = ctx.enter_context(tc.tile_pool(name="sgate", bufs=2))
        hp = ctx.enter_context(tc.tile_pool(name="hstate", bufs=2))
        cp = ctx.enter_context(tc.tile_pool(name="cstate", bufs=1))
        tp = ctx.enter_context(tc.tile_pool(name="tmp", bufs=2))
        op = ctx.enter_context(tc.tile_pool(name="outbuf", bufs=1))
        pg = ctx.enter_context(tc.tile_pool(name="psum_g", bufs=2, space="PSUM"))
        pm = ctx.enter_context(tc.tile_pool(name="psum_m", bufs=2, space="PSUM"))
        py = ctx.enter_context(tc.tile_pool(name="psum_y", bufs=1, space="PSUM"))
        pd = ctx.enter_context(tc.tile_pool(name="psum_d", bufs=1, space="PSUM"))

        # --- constants into SBUF
        whh = wp.tile([128, 128], F32)
        nc.sync.dma_start(whh[:], d_whh[:])
        wx1d = wp.tile([128, 128], F32)
        nc.sync.dma_start(wx1d[:], d_wx1d[:])
        wm1 = wp.tile([128, 32], F32)
        nc.sync.dma_start(wm1[:], d_wm1[:])
        wm2 = wp.tile([128, 1], F32)
        nc.sync.dma_start(wm2[:], d_wm2[:])
        b1c = wp.tile([128, 1], F32)
        nc.sync.dma_start(b1c[:], d_b1[:])
        eye = wp.tile([128, TAU * 32], F32)
        nc.sync.dma_start(eye[:], d_eye[:])
        b2h = wp.tile([128, 1], F32)
        nc.sync.dma_start(b2h[:], d_b2h[:])

        # --- state
        cst = cp.tile([128, N], F32)
        nc.vector.memset(cst[:], 0.0)
        hst = cp.tile([128, N], F32)
        nc.vector.memset(hst[:], 0.0)
        out_t = op.tile([128, (T // TAU) * N], F32)
        dtail = cp.tile([128, N], F32)

        # --- x block tiles, created on demand (prefetched one block early)
        zb_tiles = {}

        def get_zb(blk):
            if blk not in zb_tiles:
                zt = zp.tile([128, SB * N], F32, tag="zb")
                for j in range(4):
                    nc.sync.dma_start(zt[32 * j + 1 : 32 * j + 6, :], d_x[blk, j])
                zb_tiles[blk] = zt
                if len(zb_tiles) > 3:
                    del zb_tiles[min(zb_tiles)]
            return zb_tiles[blk]

        z0 = get_zb(0)
        # d(t=0) stored value: tanh form of d=0 is -1
        z0v = z0[:].rearrange("(a p) (s n) -> a p s n", p=32, n=N)
        if use_strided:
            nc.vector.memset(z0v[:, 0, 0], -1.0)
        else:
            for j in range(4):
                nc.vector.memset(z0[32 * j : 32 * j + 1, 0:N], -1.0)

        dsp = None
        for t in range(T):
            blk, s = divmod(t, SB)
            zbt = get_zb(blk)
            if s == 0 and blk + 1 < T // SB:
                get_zb(blk + 1)  # prefetch next x block
            if t + 1 < T:
                nblk, ns = divmod(t + 1, SB)
                zbn, nscol = get_zb(nblk), ns
            else:
                zbn, nscol = dtail, 0

            ep, tau = divmod(t, TAU)

            # ---- gates: G[128, 4N] cols (g,i,f,o)
            G = pg.tile([128, 4 * N], F32, tag="G")
            for j in range(4):
                r = 32 * j
                for g in range(4):
                    gc = slice(g * N, (g + 1) * N)
                    nc.tensor.matmul(
                        G[r : r + 32, gc],
                        wx1d[r : r + 6, g * 32 : g * 32 + 32],
                        zbt[r : r + 6, s * N : (s + 1) * N],
                        start=True, stop=False, tile_position=(r, r),
                    )
                    nc.tensor.matmul(
                        G[r : r + 32, gc],
                        whh[r : r + 32, g * 32 : g * 32 + 32],
                        hst[r : r + 32, :],
                        start=False, stop=True, tile_position=(r, r),
                    )

            # ---- activations on gates
            S = sp.tile([128, 4 * N], F32, tag="S")
            nc.scalar.activation(S[:, 0:N], G[:, 0:N], AF.Tanh)
            nc.scalar.activation(S[:, N : 4 * N], G[:, N : 4 * N], AF.Sigmoid)

            # ---- c, h update   (cols: 0:N=tg, N:2N=si, 2N:3N=sf, 3N:4N=so)
            q = tp.tile([128, N], F32, tag="q")
            nc.vector.tensor_mul(q[:], S[:, 2 * N : 3 * N], cst[:])
            pp = tp.tile([128, N], F32, tag="p")
            nc.vector.tensor_mul(pp[:], S[:, N : 2 * N], S[:, 0:N])
            nc.vector.tensor_add(cst[:], q[:], pp[:])
            th = tp.tile([128, N], F32, tag="th")
            nc.scalar.activation(th[:], cst[:], AF.Tanh)
            nc.vector.tensor_mul(hst[:], S[:, 3 * N : 4 * N], th[:])

            # ---- decision MLP
            M1 = pm.tile([128, N], F32, tag="M1")
            for j in range(4):
                r = 32 * j
                nc.tensor.matmul(M1[r : r + 32, :], wm1[r : r + 32, :],
                                 hst[r : r + 32, :], start=True, stop=True,
                                 tile_position=(r, r))
            R = tp.tile([128, N], F32, tag="R")
            nc.scalar.activation(R[:], M1[:], AF.Relu, bias=b1c[:, 0:1])
            Y = py.tile([128, N], F32, tag="Y")
            for j in range(4):
                r = 32 * j
                nc.tensor.matmul(Y[r : r + 1, :], wm2[r : r + 32, 0:1],
                                 R[r : r + 32, :], start=True, stop=True,
                                 tile_position=(r, r))

            # ---- d = tanh(0.5 y + 0.5 b2)  (stored tanh form)
            znv = zbn[:].rearrange("(a p) (s n) -> a p s n", p=32, n=N)
            if use_strided:
                yv = Y[:].rearrange("(a p) n -> a p n", p=32)
                b2v = b2h[:].rearrange("(a p) n -> a p n", p=32)
                nc.scalar.activation(znv[:, 0, nscol], yv[:, 0], AF.Tanh,
                                     bias=b2v[:, 0], scale=0.5)
            else:
                for j in range(4):
                    r = 32 * j
                    nc.scalar.activation(
                        zbn[r : r + 1, nscol * N : (nscol + 1) * N],
                        Y[r : r + 1, :], AF.Tanh,
                        bias=b2h[r : r + 1, 0:1], scale=0.5)

            # ---- spread d across partitions for the output
            if tau == 0:
                dsp = pd.tile([128, N], F32, tag="D")
            for j in range(4):
                r = 32 * j
                nc.tensor.matmul(
                    dsp[r : r + 32, :],
                    eye[r : r + 1, tau * 32 : tau * 32 + 32],
                    zbn[r : r + 1, nscol * N : (nscol + 1) * N],
                    start=(tau == 0), stop=(tau == TAU - 1),
                    skip_group_check=True, tile_position=(r, r),
                )
            if tau == TAU - 1:
                nc.vector.transpose(out_t[:, ep * N : (ep + 1) * N], dsp[:])

        # ---- final affine sigma = 0.5 + 0.5 tanh  and store
        nc.vector.tensor_scalar(out_t[:], out_t[:], 0.5, 0.5, ALU.mult, ALU.add)
        ov = out_t[:].rearrange("p (e nb tt) -> p e nb tt", e=T // TAU, tt=32)
        dov = d_out[:].rearrange("(a nb pp) (e tt) -> a nb pp e tt",
                                 a=4, pp=32, tt=32)
        for j in range(4):
            for nb in range(N // 32):
                nc.sync.dma_start(dov[j, nb], ov[32 * j : 32 * j + 32, :, nb])

    return d_out


# ----------------------------------------------------------------------------
# numpy reference of the exact kernel math (for mini-tests)
# ----------------------------------------------------------------------------
def numpy_model(x, params):
    """x [B, T, 4] -> [B, T] float32, same math as reference()."""
    import jax
    import jax.numpy as jnp
    B, T, F = x.shape
    H = params["W_hh"].shape[1]
    inv_std = 1.0 / np.sqrt(params["bn_var"] + EPS)

    h = np.zeros((B, H), np.float32)
    c = np.zeros((B, H), np.float32)
    d = np.zeros((B, 1), np.float32)
    outs = np.zeros((B, T), np.float32)
    sig = lambda v: 1.0 / (1.0 + np.exp(-v))
    for t in range(T):
        z = np.concatenate([x[:, t], d], 1)
        z = (z - params["bn_mean"]) * inv_std * params["bn_gamma"] + params["bn_beta"]
        gates = z @ params["W_ih"].T + params["b_ih"] + h @ params["W_hh"].T + params["b_hh"]
        i, f, g, o = np.split(gates, 4, 1)
        c = sig(f) * c + sig(i) * np.tanh(g)
        h = sig(o) * np.tanh(c)
        d = sig(np.maximum(h @ params["W1"].T + params["b1"], 0) @ params["W2"].T + params["b2"])
        outs[:, t] = d[:, 0]
    return outs


# ----------------------------------------------------------------------------
# Entry point
# ----------------------------------------------------------------------------
_CACHE = {}


def _get_compiled():
    if "nc" not in _CACHE:
        nc = bacc.Bacc("TRN2", target_bir_lowering=False, debug=False)
        build_kernel(nc, FULL)
        nc.compile()
        _CACHE["nc"] = nc
    return _CACHE["nc"]


def kernel(**inputs):
    from concourse.bass_utils import run_bass_kernel_spmd

    x = np.asarray(inputs["x"], np.float32)
    B, T, F = x.shape
    ncores = 8
    bc = B // ncores
    folded = fold_params(inputs)
    eye = make_eye(FULL)
    b2h = np.full((128, 1), folded["b2half"], np.float32)

    nc = _get_compiled()
    in_maps = []
    for c in range(ncores):
        m = {
            "xprep": prep_x(x[c * bc : (c + 1) * bc], FULL),
            "whh": folded["whh"],
            "wx1d": folded["wx1d"],
            "wm1": folded["wm1"],
            "wm2": folded["wm2"],
            "b1col": folded["b1col"],
            "eye": eye,
            "b2half": b2h,
        }
        in_maps.append(m)

    res = run_bass_kernel_spmd(nc, in_maps, list(range(ncores)))
    outs = [res.results[c]["dout"] for c in range(ncores)]
    return np.concatenate(outs, 0)[:, :, None].astype(np.float32)


# revision 20
# speedup vs baseline: 1.0687x; 1.0687x over previous
"""Trainium2 Bass kernel for DeepHedgingModel (LSTM scan, B=8192 T=512 F=4 H=32).

Strategy (pure data parallel over 8 cores, 1024 batch rows per core):

Per core, the 1024-row batch is split into 4 "bands" of 256 columns; band j
owns SBUF/PSUM partitions [32j, 32j+32).  All per-gate matmuls are M=32
tile_position matmuls whose outputs land inside the band, so every
elementwise LSTM op is partition-aligned and runs over all 4 bands in a
single [128, 256] instruction.

Gate-type column order in G is (g, i, f, o) so one Tanh covers cols 0:256
and one Sigmoid covers cols 256:1024.

z-vector per band = rows [d, x0..x3, ones] at partitions 32j..32j+6.
BatchNorm (inference affine) is folded into the stationary weights; the
d-feedback sigmoid is computed as tanh: sigma(y) = 0.5 + 0.5*tanh(y/2),
and the 0.5/0.5 affine is folded into the d-row weights + bias row, so the
raw tanh output is stored as the recurrent "d" row.  The output path
spreads d over partitions with a one-hot e_tau x d rank-1 matmul
accumulated in PSUM over 32 steps, then a DVE 32x32 stream-transpose moves
batch onto partitions; the 0.5+0.5x affine is applied once at the end.
"""

import os
import sys
from contextlib import ExitStack

import numpy as np

sys.path.insert(0, "/opt/trn_rl_repo")

import concourse.bass as bass  # noqa: E402
import concourse.tile as tile  # noqa: E402
from concourse import bacc, mybir  # noqa: E402

F32 = mybir.dt.float32
F32R = mybir.dt.float32r
AF = mybir.ActivationFunctionType
ALU = mybir.AluOpType


def _r(ap):
    """View an fp32 AP as float32r for the fast PE path (same bits)."""
    return ap.bitcast(F32R)

EPS = 1e-5


# ----------------------------------------------------------------------------
# Config
# ----------------------------------------------------------------------------
class Cfg:
    def __init__(self, ncol=256, T=512, sblk=16, tau=32, nbands=4):
        self.ncol = ncol          # batch columns per band
        self.T = T                # timesteps
        self.sblk = sblk          # steps per x-DMA block
        self.tau = tau            # steps per d-spread epoch (<= 32)
        self.nbands = nbands      # 4 bands of 32 partitions
        self.B = nbands * ncol    # per-core batch
        assert T % sblk == 0 and T % tau == 0


FULL = Cfg()


# ----------------------------------------------------------------------------
# Host-side weight folding / input prep
# ----------------------------------------------------------------------------
def fold_params(p):
    """Return dict of numpy arrays for the SBUF-resident constants."""
    H = 32
    # gate-type order in G columns: (g, i, f, o); torch rows are (i, f, g, o)
    perm = np.concatenate([
        np.arange(2 * H, 3 * H),   # g
        np.arange(0, H),           # i
        np.arange(H, 2 * H),       # f
        np.arange(3 * H, 4 * H),   # o
    ])
    W_ih = p["W_ih"].astype(np.float64)
    W_hh = p["W_hh"].astype(np.float64)
    b_ih = p["b_ih"].astype(np.float64)
    b_hh = p["b_hh"].astype(np.float64)
    gam = p["bn_gamma"].astype(np.float64)
    bet = p["bn_beta"].astype(np.float64)
    mu = p["bn_mean"].astype(np.float64)
    var = p["bn_var"].astype(np.float64)
    a = gam / np.sqrt(var + EPS)          # [5]
    b_a = bet - mu * a                    # [5]

    Wx_eff = W_ih[:, :4] * a[None, :4]    # [128, 4]
    w_d_eff = W_ih[:, 4] * a[4]           # [128]
    C = b_a @ W_ih.T + b_ih + b_hh        # [128]
    # recurrence stores t_y = tanh(y/2);  d = 0.5 + 0.5 t_y
    d_row = 0.5 * w_d_eff                 # weight on stored t_y
    C = C + 0.5 * w_d_eff                 # constant part of d contribution

    # scale the tanh-gate (g) rows?  no: direct tanh table is used.
    out = {}
    # Whh_sb [128, 128]: [32j+k, 32g+m] = W_hh[perm[32g+m], k]
    Whh_band = W_hh[perm, :].T            # [k 32, 128 gates(permuted)]
    out["whh"] = np.tile(Whh_band[:, :], (4, 1)).astype(np.float32)  # [128,128]
    # Wx1d_sb [128, 128]: band rows r: 0=d_row, 1..4=x, 5=ones(C)
    wx1d_band = np.zeros((32, 128))
    wx1d_band[0, :] = d_row[perm]
    wx1d_band[1:5, :] = Wx_eff[perm, :].T
    wx1d_band[5, :] = C[perm]
    out["wx1d"] = np.tile(wx1d_band, (4, 1)).astype(np.float32)      # [128,128]
    # Wm1_sb [128, 32]: [32j+k, m] = W1[m, k]
    W1 = p["W1"].astype(np.float64)
    out["wm1"] = np.tile(W1.T, (4, 1)).astype(np.float32)            # [128,32]
    # Wm2_sb [128, 1]: [32j+k, 0] = W2[0, k]
    W2 = p["W2"].astype(np.float64)
    out["wm2"] = np.tile(W2.T, (4, 1)).astype(np.float32)            # [128,1]
    # per-partition b1 bias column [128,1]
    out["b1col"] = np.tile(p["b1"].astype(np.float32)[:, None], (4, 1))
    out["b2half"] = float(0.5 * p["b2"].astype(np.float64)[0])
    return out


def make_eye(cfg):
    """E_sb [128, tau*32]: row 32j holds one-hot e_tau blocks."""
    E = np.zeros((128, cfg.tau * 32), np.float32)
    for t in range(cfg.tau):
        E[::32, t * 32 + t] = 1.0
    return E


def prep_x(x_core, cfg):
    """x_core [B, T, 4] -> xprep [T/sblk, nbands, 5, sblk, ncol] (ones in row 4)."""
    B, T, F = x_core.shape
    nb, nc_, sb = cfg.nbands, cfg.ncol, cfg.sblk
    xp = np.empty((T // sb, nb, 5, sb, nc_), np.float32)
    # x_core[b, t, f] with b = j*ncol + n
    xr = x_core.reshape(nb, nc_, T // sb, sb, F)
    xp[:, :, :4] = xr.transpose(2, 0, 4, 3, 1)  # [blk, band, f, s, n]
    xp[:, :, 4] = 1.0
    return xp


# ----------------------------------------------------------------------------
# Kernel body
# ----------------------------------------------------------------------------
def build_kernel(nc, cfg, use_strided=False, time_mode=False):
    """Declare DRAM I/O and emit the TileContext program.

    time_mode=True shrinks xprep to one block that every step re-reads —
    wrong math, identical instruction stream — to measure device time
    without the axon per-call input-transfer cost.
    """
    N = cfg.ncol
    T, SB, TAU = cfg.T, cfg.sblk, cfg.tau

    nxblk = 1 if time_mode else T // SB
    d_x = nc.dram_tensor("xprep", [nxblk, 4, 5, SB, N], F32, kind="ExternalInput")
    d_whh = nc.dram_tensor("whh", [128, 128], F32, kind="ExternalInput")
    d_wx1d = nc.dram_tensor("wx1d", [128, 128], F32, kind="ExternalInput")
    d_wm1 = nc.dram_tensor("wm1", [128, 32], F32, kind="ExternalInput")
    d_wm2 = nc.dram_tensor("wm2", [128, 1], F32, kind="ExternalInput")
    d_b1 = nc.dram_tensor("b1col", [128, 1], F32, kind="ExternalInput")
    d_eye = nc.dram_tensor("eye", [128, TAU * 32], F32, kind="ExternalInput")
    d_b2h = nc.dram_tensor("b2half", [128, 1], F32, kind="ExternalInput")
    d_out = nc.dram_tensor("dout", [cfg.B, T], F32, kind="ExternalOutput")

    with tile.TileContext(nc) as tc, ExitStack() as ctx:
        wp = ctx.enter_context(tc.tile_pool(name="weights", bufs=1))
        zp = ctx.enter_context(tc.tile_pool(name="zb", bufs=3))
        sp = ctx.enter_context(tc.tile_pool(name="sgate", bufs=2))
        hp = ctx.enter_context(tc.tile_pool(name="hstate", bufs=2))
        cp = ctx.enter_context(tc.tile_pool(name="cstate", bufs=1))
        tp = ctx.enter_context(tc.tile_pool(name="tmp", bufs=2))
        op = ctx.enter_context(tc.tile_pool(name="outbuf", bufs=1))
        pg = ctx.enter_context(tc.tile_pool(name="psum_g", bufs=2, space="PSUM"))
        pm = ctx.enter_context(tc.tile_pool(name="psum_m", bufs=2, space="PSUM"))
        py = ctx.enter_context(tc.tile_pool(name="psum_y", bufs=1, space="PSUM"))
        pd = ctx.enter_context(tc.tile_pool(name="psum_d", bufs=1, space="PSUM"))

        # --- constants into SBUF
        whh = wp.tile([128, 128], F32)
        nc.sync.dma_start(whh[:], d_whh[:])
        wx1d = wp.tile([128, 128], F32)
        nc.sync.dma_start(wx1d[:], d_wx1d[:])
        wm1 = wp.tile([128, 32], F32)
        nc.sync.dma_start(wm1[:], d_wm1[:])
        wm2 = wp.tile([128, 1], F32)
        nc.sync.dma_start(wm2[:], d_wm2[:])
        b1c = wp.tile([128, 1], F32)
        nc.sync.dma_start(b1c[:], d_b1[:])
        eye = wp.tile([128, TAU * 32], F32)
        nc.sync.dma_start(eye[:], d_eye[:])
        b2h = wp.tile([128, 1], F32)
        nc.sync.dma_start(b2h[:], d_b2h[:])

        # --- state
        cst = cp.tile([128, N], F32)
        nc.vector.memset(cst[:], 0.0)
        hst = cp.tile([128, N], F32)
        nc.vector.memset(hst[:], 0.0)
        out_t = op.tile([128, (T // TAU) * N], F32)
        dtail = cp.tile([128, N], F32)

        # --- x block tiles, created on demand (prefetched one block early)
        zb_tiles = {}

        def get_zb(blk):
            if blk not in zb_tiles:
                zt = zp.tile([128, SB * N], F32, tag="zb")
                xblk = 0 if time_mode else blk
                for j in range(4):
                    nc.sync.dma_start(zt[32 * j + 1 : 32 * j + 6, :], d_x[xblk, j])
                zb_tiles[blk] = zt
                if len(zb_tiles) > 3:
                    del zb_tiles[min(zb_tiles)]
            return zb_tiles[blk]

        z0 = get_zb(0)
        # d(t=0) stored value: tanh form of d=0 is -1
        z0v = z0[:].rearrange("(a p) (s n) -> a p s n", p=32, n=N)
        if use_strided:
            nc.vector.memset(z0v[:, 0, 0], -1.0)
        else:
            for j in range(4):
                nc.vector.memset(z0[32 * j : 32 * j + 1, 0:N], -1.0)

        dsp = None
        for t in range(T):
            blk, s = divmod(t, SB)
            zbt = get_zb(blk)
            if s == 0 and blk + 1 < T // SB:
                get_zb(blk + 1)  # prefetch next x block
            if t + 1 < T:
                nblk, ns = divmod(t + 1, SB)
                zbn, nscol = get_zb(nblk), ns
            else:
                zbn, nscol = dtail, 0

            ep, tau = divmod(t, TAU)

            # ---- gates: G[128, 4N] cols (g,i,f,o)
            G = pg.tile([128, 4 * N], F32, tag="G")
            for j in range(4):
                r = 32 * j
                for g in range(4):
                    gc = slice(g * N, (g + 1) * N)
                    nc.tensor.matmul(
                        G[r : r + 32, gc],
                        wx1d[r : r + 6, g * 32 : g * 32 + 32],
                        zbt[r : r + 6, s * N : (s + 1) * N],
                        start=True, stop=False, tile_position=(r, r),
                    )
                    nc.tensor.matmul(
                        G[r : r + 32, gc],
                        whh[r : r + 32, g * 32 : g * 32 + 32],
                        hst[r : r + 32, :],
                        start=False, stop=True, tile_position=(r, r),
                    )

            # ---- activations on gates
            S = sp.tile([128, 4 * N], F32, tag="S")
            nc.scalar.activation(S[:, 0:N], G[:, 0:N], AF.Tanh)
            nc.scalar.activation(S[:, N : 4 * N], G[:, N : 4 * N], AF.Sigmoid)

            # ---- c, h update   (cols: 0:N=tg, N:2N=si, 2N:3N=sf, 3N:4N=so)
            q = tp.tile([128, N], F32, tag="q")
            nc.vector.tensor_mul(q[:], S[:, 2 * N : 3 * N], cst[:])
            pp = tp.tile([128, N], F32, tag="p")
            nc.vector.tensor_mul(pp[:], S[:, N : 2 * N], S[:, 0:N])
            nc.vector.tensor_add(cst[:], q[:], pp[:])
            th = tp.tile([128, N], F32, tag="th")
            nc.scalar.activation(th[:], cst[:], AF.Tanh)
            nc.vector.tensor_mul(hst[:], S[:, 3 * N : 4 * N], th[:])

            # ---- decision MLP
            M1 = pm.tile([128, N], F32, tag="M1")
            for j in range(4):
                r = 32 * j
                nc.tensor.matmul(M1[r : r + 32, :], wm1[r : r + 32, :],
                                 hst[r : r + 32, :], start=True, stop=True,
                                 tile_position=(r, r))
            R = tp.tile([128, N], F32, tag="R")
            nc.scalar.activation(R[:], M1[:], AF.Relu, bias=b1c[:, 0:1])
            Y = py.tile([128, N], F32, tag="Y")
            for j in range(4):
                r = 32 * j
                nc.tensor.matmul(Y[r : r + 1, :], wm2[r : r + 32, 0:1],
                                 R[r : r + 32, :], start=True, stop=True,
                                 tile_position=(r, r))

            # ---- d = tanh(0.5 y + 0.5 b2)  (stored tanh form)
            znv = zbn[:].rearrange("(a p) (s n) -> a p s n", p=32, n=N)
            if use_strided:
                yv = Y[:].rearrange("(a p) n -> a p n", p=32)
                b2v = b2h[:].rearrange("(a p) n -> a p n", p=32)
                nc.scalar.activation(znv[:, 0, nscol], yv[:, 0], AF.Tanh,
                                     bias=b2v[:, 0], scale=0.5)
            else:
                for j in range(4):
                    r = 32 * j
                    nc.scalar.activation(
                        zbn[r : r + 1, nscol * N : (nscol + 1) * N],
                        Y[r : r + 1, :], AF.Tanh,
                        bias=b2h[r : r + 1, 0:1], scale=0.5)

            # ---- spread d across partitions for the output
            if tau == 0:
                dsp = pd.tile([128, N], F32, tag="D")
            for j in range(4):
                r = 32 * j
                nc.tensor.matmul(
                    dsp[r : r + 32, :],
                    eye[r : r + 1, tau * 32 : tau * 32 + 32],
                    zbn[r : r + 1, nscol * N : (nscol + 1) * N],
                    start=(tau == 0), stop=(tau == TAU - 1),
                    skip_group_check=True, tile_position=(r, r),
                )
            if tau == TAU - 1:
                nc.vector.transpose(out_t[:, ep * N : (ep + 1) * N], dsp[:])

        # ---- final affine sigma = 0.5 + 0.5 tanh  and store
        nc.vector.tensor_scalar(out_t[:], out_t[:], 0.5, 0.5, ALU.mult, ALU.add)
        ov = out_t[:].rearrange("p (e nb tt) -> p e nb tt", e=T // TAU, tt=32)
        dov = d_out[:].rearrange("(a nb pp) (e tt) -> a nb pp e tt",
                                 a=4, pp=32, tt=32)
        for j in range(4):
            for nb in range(N // 32):
                nc.sync.dma_start(dov[j, nb], ov[32 * j : 32 * j + 32, :, nb])

    return d_out


# ----------------------------------------------------------------------------
# numpy reference of the exact kernel math (for mini-tests)
# ----------------------------------------------------------------------------
def numpy_model(x, params):
    """x [B, T, 4] -> [B, T] float32, same math as reference()."""
    import jax
    import jax.numpy as jnp
    B, T, F = x.shape
    H = params["W_hh"].shape[1]
    inv_std = 1.0 / np.sqrt(params["bn_var"] + EPS)

    h = np.zeros((B, H), np.float32)
    c = np.zeros((B, H), np.float32)
    d = np.zeros((B, 1), np.float32)
    outs = np.zeros((B, T), np.float32)
    sig = lambda v: 1.0 / (1.0 + np.exp(-v))
    for t in range(T):
        z = np.concatenate([x[:, t], d], 1)
        z = (z - params["bn_mean"]) * inv_std * params["bn_gamma"] + params["bn_beta"]
        gates = z @ params["W_ih"].T + params["b_ih"] + h @ params["W_hh"].T + params["b_hh"]
        i, f, g, o = np.split(gates, 4, 1)
        c = sig(f) * c + sig(i) * np.tanh(g)
        h = sig(o) * np.tanh(c)
        d = sig(np.maximum(h @ params["W1"].T + params["b1"], 0) @ params["W2"].T + params["b2"])
        outs[:, t] = d[:, 0]
    return outs


# ----------------------------------------------------------------------------
# Entry point
# ----------------------------------------------------------------------------
_CACHE = {}


def _get_compiled():
    if "nc" not in _CACHE:
        nc = bacc.Bacc("TRN2", target_bir_lowering=False, debug=False)
        build_kernel(nc, FULL)
        nc.compile()
        _CACHE["nc"] = nc
    return _CACHE["nc"]


def kernel(**inputs):
    from concourse.bass_utils import run_bass_kernel_spmd

    x = np.asarray(inputs["x"], np.float32)
    B, T, F = x.shape
    ncores = 8
    bc = B // ncores
    folded = fold_params(inputs)
    eye = make_eye(FULL)
    b2h = np.full((128, 1), folded["b2half"], np.float32)

    nc = _get_compiled()
    in_maps = []
    for c in range(ncores):
        m = {
            "xprep": prep_x(x[c * bc : (c + 1) * bc], FULL),
            "whh": folded["whh"],
            "wx1d": folded["wx1d"],
            "wm1": folded["wm1"],
            "wm2": folded["wm2"],
            "b1col": folded["b1col"],
            "eye": eye,
            "b2half": b2h,
        }
        in_maps.append(m)

    res = run_bass_kernel_spmd(nc, in_maps, list(range(ncores)))
    outs = [res.results[c]["dout"] for c in range(ncores)]
    return np.concatenate(outs, 0)[:, :, None].astype(np.float32)


# revision 34
# speedup vs baseline: 1.1611x; 1.0864x over previous
"""Trainium2 Bass kernel for DeepHedgingModel (LSTM scan, B=8192 T=512 F=4 H=32).

Strategy (pure data parallel over 8 cores, 1024 batch rows per core):

Per core, the 1024-row batch is split into 4 "bands" of 256 columns; band j
owns SBUF/PSUM partitions [32j, 32j+32).  All per-gate matmuls are M=32
tile_position matmuls whose outputs land inside the band, so every
elementwise LSTM op is partition-aligned and runs over all 4 bands in a
single [128, 256] instruction.

Gate-type column order in G is (g, i, f, o) so one Tanh covers cols 0:256
and one Sigmoid covers cols 256:1024.

z-vector per band = rows [d, x0..x3, ones] at partitions 32j..32j+6.
BatchNorm (inference affine) is folded into the stationary weights; the
d-feedback sigmoid is computed as tanh: sigma(y) = 0.5 + 0.5*tanh(y/2),
and the 0.5/0.5 affine is folded into the d-row weights + bias row, so the
raw tanh output is stored as the recurrent "d" row.  The output path
spreads d over partitions with a one-hot e_tau x d rank-1 matmul
accumulated in PSUM over 32 steps, then a DVE 32x32 stream-transpose moves
batch onto partitions; the 0.5+0.5x affine is applied once at the end.
"""

import sys
from contextlib import ExitStack

import numpy as np

sys.path.insert(0, "/opt/trn_rl_repo")

import concourse.tile as tile  # noqa: E402
from concourse import bacc, mybir  # noqa: E402

F32 = mybir.dt.float32
AF = mybir.ActivationFunctionType
ALU = mybir.AluOpType

EPS = 1e-5


# ----------------------------------------------------------------------------
# Config
# ----------------------------------------------------------------------------
class Cfg:
    def __init__(self, ncol=256, T=512, sblk=16, tau=32, nbands=4):
        self.ncol = ncol          # batch columns per band
        self.T = T                # timesteps
        self.sblk = sblk          # steps per x-DMA block
        self.tau = tau            # steps per d-spread epoch (<= 32)
        self.nbands = nbands      # 4 bands of 32 partitions
        self.B = nbands * ncol    # per-core batch
        assert T % sblk == 0 and T % tau == 0


FULL = Cfg()


# ----------------------------------------------------------------------------
# Host-side weight folding / input prep
# ----------------------------------------------------------------------------
def fold_params(p):
    """Return dict of numpy arrays for the SBUF-resident constants."""
    H = 32
    # gate-type order in G columns: (g, i, f, o); torch rows are (i, f, g, o)
    perm = np.concatenate([
        np.arange(2 * H, 3 * H),   # g
        np.arange(0, H),           # i
        np.arange(H, 2 * H),       # f
        np.arange(3 * H, 4 * H),   # o
    ])
    W_ih = p["W_ih"].astype(np.float64)
    W_hh = p["W_hh"].astype(np.float64)
    b_ih = p["b_ih"].astype(np.float64)
    b_hh = p["b_hh"].astype(np.float64)
    gam = p["bn_gamma"].astype(np.float64)
    bet = p["bn_beta"].astype(np.float64)
    mu = p["bn_mean"].astype(np.float64)
    var = p["bn_var"].astype(np.float64)
    a = gam / np.sqrt(var + EPS)          # [5]
    b_a = bet - mu * a                    # [5]

    Wx_eff = W_ih[:, :4] * a[None, :4]    # [128, 4]
    w_d_eff = W_ih[:, 4] * a[4]           # [128]
    C = b_a @ W_ih.T + b_ih + b_hh        # [128]
    # recurrence stores t_y = tanh(y/2);  d = 0.5 + 0.5 t_y
    d_row = 0.5 * w_d_eff                 # weight on stored t_y
    C = C + 0.5 * w_d_eff                 # constant part of d contribution

    # scale the tanh-gate (g) rows?  no: direct tanh table is used.
    out = {}
    # Whh_sb [128, 128]: [32j+k, 32g+m] = W_hh[perm[32g+m], k]
    Whh_band = W_hh[perm, :].T            # [k 32, 128 gates(permuted)]
    out["whh"] = np.tile(Whh_band[:, :], (4, 1)).astype(np.float32)  # [128,128]
    # Wx1d_sb [128, 128]: band rows r: 0=d_row, 1..4=x, 5=ones(C)
    wx1d_band = np.zeros((32, 128))
    wx1d_band[0, :] = d_row[perm]
    wx1d_band[1:5, :] = Wx_eff[perm, :].T
    wx1d_band[5, :] = C[perm]
    out["wx1d"] = np.tile(wx1d_band, (4, 1)).astype(np.float32)      # [128,128]
    # Wm1_sb [128, 32]: [32j+k, m] = W1[m, k]
    W1 = p["W1"].astype(np.float64)
    out["wm1"] = np.tile(W1.T, (4, 1)).astype(np.float32)            # [128,32]
    # Wm2_sb [128, 1]: [32j+k, 0] = W2[0, k]
    W2 = p["W2"].astype(np.float64)
    out["wm2"] = np.tile(W2.T, (4, 1)).astype(np.float32)            # [128,1]
    # per-partition b1 bias column [128,1]
    out["b1col"] = np.tile(p["b1"].astype(np.float32)[:, None], (4, 1))
    out["b2half"] = float(0.5 * p["b2"].astype(np.float64)[0])
    return out


def make_eye(cfg):
    """E_sb [128, tau*32]: row 32j holds one-hot e_tau blocks."""
    E = np.zeros((128, cfg.tau * 32), np.float32)
    for t in range(cfg.tau):
        E[::32, t * 32 + t] = 1.0
    return E


def prep_x(x_core, cfg):
    """x_core [B, T, 4] -> xprep [T/sblk, nbands, 5, sblk, ncol] (ones in row 4)."""
    B, T, F = x_core.shape
    nb, nc_, sb = cfg.nbands, cfg.ncol, cfg.sblk
    xp = np.empty((T // sb, nb, 5, sb, nc_), np.float32)
    # x_core[b, t, f] with b = j*ncol + n
    xr = x_core.reshape(nb, nc_, T // sb, sb, F)
    xp[:, :, :4] = xr.transpose(2, 0, 4, 3, 1)  # [blk, band, f, s, n]
    xp[:, :, 4] = 1.0
    return xp


# ----------------------------------------------------------------------------
# Kernel body
# ----------------------------------------------------------------------------
def build_kernel(nc, cfg, use_strided=False, time_mode=False):
    """Declare DRAM I/O and emit the TileContext program.

    time_mode=True shrinks xprep to one block that every step re-reads —
    wrong math, identical instruction stream — to measure device time
    without the axon per-call input-transfer cost.
    """
    N = cfg.ncol
    T, SB, TAU = cfg.T, cfg.sblk, cfg.tau

    nxblk = 1 if time_mode else T // SB
    d_x = nc.dram_tensor("xprep", [nxblk, 4, 5, SB, N], F32, kind="ExternalInput")
    d_whh = nc.dram_tensor("whh", [128, 128], F32, kind="ExternalInput")
    d_wx1d = nc.dram_tensor("wx1d", [128, 128], F32, kind="ExternalInput")
    d_wm1 = nc.dram_tensor("wm1", [128, 32], F32, kind="ExternalInput")
    d_wm2 = nc.dram_tensor("wm2", [128, 1], F32, kind="ExternalInput")
    d_b1 = nc.dram_tensor("b1col", [128, 1], F32, kind="ExternalInput")
    d_b2h = nc.dram_tensor("b2half", [128, 1], F32, kind="ExternalInput")
    # raw tanh(y/2) history, dumped per x-block; host does transpose + affine
    d_out = nc.dram_tensor("d_scratch", [T // SB + 1, 4, SB * N], F32,
                           kind="ExternalOutput")

    with tile.TileContext(nc) as tc, ExitStack() as ctx:
        wp = ctx.enter_context(tc.tile_pool(name="weights", bufs=1))
        zp = ctx.enter_context(tc.tile_pool(name="zb", bufs=3))
        sp = ctx.enter_context(tc.tile_pool(name="sgate", bufs=2))
        hp = ctx.enter_context(tc.tile_pool(name="hstate", bufs=2))
        cp = ctx.enter_context(tc.tile_pool(name="cstate", bufs=1))
        tp = ctx.enter_context(tc.tile_pool(name="tmp", bufs=2))
        pg = ctx.enter_context(tc.tile_pool(name="psum_g", bufs=2, space="PSUM"))
        pm = ctx.enter_context(tc.tile_pool(name="psum_m", bufs=1, space="PSUM"))
        py = ctx.enter_context(tc.tile_pool(name="psum_y", bufs=1, space="PSUM"))

        # --- constants into SBUF
        whh = wp.tile([128, 128], F32)
        nc.sync.dma_start(whh[:], d_whh[:])
        wx1d = wp.tile([128, 128], F32)
        nc.sync.dma_start(wx1d[:], d_wx1d[:])
        wm1 = wp.tile([128, 32], F32)
        nc.sync.dma_start(wm1[:], d_wm1[:])
        wm2 = wp.tile([128, 1], F32)
        nc.sync.dma_start(wm2[:], d_wm2[:])
        b1c = wp.tile([128, 1], F32)
        nc.sync.dma_start(b1c[:], d_b1[:])
        b2h = wp.tile([128, 1], F32)
        nc.sync.dma_start(b2h[:], d_b2h[:])

        # --- state
        cst = cp.tile([128, N], F32)
        nc.vector.memset(cst[:], 0.0)
        hst = cp.tile([128, N], F32)
        nc.vector.memset(hst[:], 0.0)
        dtail = cp.tile([128, N], F32)

        # --- x block tiles, created on demand (prefetched one block early)
        zb_tiles = {}

        def get_zb(blk):
            if blk not in zb_tiles:
                zt = zp.tile([128, SB * N], F32, tag="zb")
                xblk = 0 if time_mode else blk
                for j in range(4):
                    nc.sync.dma_start(zt[32 * j + 1 : 32 * j + 6, :], d_x[xblk, j])
                zb_tiles[blk] = zt
                if len(zb_tiles) > 3:
                    del zb_tiles[min(zb_tiles)]
            return zb_tiles[blk]

        z0 = get_zb(0)
        # d(t=0) stored value: tanh form of d=0 is -1
        z0v = z0[:].rearrange("(a p) (s n) -> a p s n", p=32, n=N)
        if use_strided:
            nc.vector.memset(z0v[:, 0, 0], -1.0)
        else:
            for j in range(4):
                nc.vector.memset(z0[32 * j : 32 * j + 1, 0:N], -1.0)

        for t in range(T):
            blk, s = divmod(t, SB)
            zbt = get_zb(blk)
            if s == 0 and blk + 1 < T // SB:
                get_zb(blk + 1)  # prefetch next x block
            if t + 1 < T:
                nblk, ns = divmod(t + 1, SB)
                zbn, nscol = get_zb(nblk), ns
            else:
                zbn, nscol = dtail, 0

            # Two independent column phases: phase ph owns batch columns
            # [c0, c0+W) of every band. The two recurrences share no data,
            # so the scheduler interleaves them and fills chain stalls.
            W = N // 2
            for ph in range(2):
                c0 = ph * W
                cs = slice(s * N + c0, s * N + c0 + W)       # zbt cols
                ns_ = slice(nscol * N + c0, nscol * N + c0 + W)  # zbn cols

                # ---- gates: G[128, 4W] cols (g,i,f,o)
                G = pg.tile([128, 4 * W], F32, tag=f"G{ph}")
                for j in range(4):
                    r = 32 * j
                    for g in range(4):
                        gc = slice(g * W, (g + 1) * W)
                        nc.tensor.matmul(
                            G[r : r + 32, gc],
                            wx1d[r : r + 6, g * 32 : g * 32 + 32],
                            zbt[r : r + 6, cs],
                            start=True, stop=False, tile_position=(r, r),
                        )
                        nc.tensor.matmul(
                            G[r : r + 32, gc],
                            whh[r : r + 32, g * 32 : g * 32 + 32],
                            hst[r : r + 32, c0 : c0 + W],
                            start=False, stop=True, tile_position=(r, r),
                        )

                # ---- activations on gates
                S = sp.tile([128, 4 * W], F32, tag=f"S{ph}")
                nc.scalar.activation(S[:, 0:W], G[:, 0:W], AF.Tanh)
                nc.scalar.activation(S[:, W : 4 * W], G[:, W : 4 * W],
                                     AF.Sigmoid)

                # ---- c, h update (cols: 0:W=tg, W:2W=si, 2W:3W=sf, 3W:4W=so)
                q = tp.tile([128, W], F32, tag=f"q{ph}")
                nc.vector.tensor_mul(q[:], S[:, 2 * W : 3 * W],
                                     cst[:, c0 : c0 + W])
                pp = tp.tile([128, W], F32, tag=f"p{ph}")
                nc.vector.tensor_mul(pp[:], S[:, W : 2 * W], S[:, 0:W])
                nc.vector.tensor_add(cst[:, c0 : c0 + W], q[:], pp[:])
                th = tp.tile([128, W], F32, tag=f"th{ph}")
                nc.scalar.activation(th[:], cst[:, c0 : c0 + W], AF.Tanh)
                nc.vector.tensor_mul(hst[:, c0 : c0 + W],
                                     S[:, 3 * W : 4 * W], th[:])

                # ---- decision MLP
                M1 = pm.tile([128, W], F32, tag=f"M1{ph}")
                for j in range(4):
                    r = 32 * j
                    nc.tensor.matmul(M1[r : r + 32, :], wm1[r : r + 32, :],
                                     hst[r : r + 32, c0 : c0 + W],
                                     start=True, stop=True,
                                     tile_position=(r, r),
                                     skip_group_check=True)
                R = tp.tile([128, W], F32, tag=f"R{ph}")
                nc.vector.tensor_scalar(R[:], M1[:], b1c[:, 0:1], 0.0,
                                        ALU.add, ALU.max)
                Y = py.tile([128, W], F32, tag=f"Y{ph}")
                for j in range(4):
                    r = 32 * j
                    nc.tensor.matmul(Y[r : r + 1, :], wm2[r : r + 32, 0:1],
                                     R[r : r + 32, :], start=True, stop=True,
                                     tile_position=(r, r),
                                     skip_group_check=True)

                # ---- d = tanh(0.5 y + 0.5 b2)  (stored tanh form)
                for j in range(4):
                    r = 32 * j
                    nc.scalar.activation(
                        zbn[r : r + 1, ns_],
                        Y[r : r + 1, :], AF.Tanh,
                        bias=b2h[r : r + 1, 0:1], scale=0.5)

            # ---- dump this block's d-rows once its last column is written
            if s == SB - 1:
                for j in range(4):
                    nc.sync.dma_start(d_out[blk, j].unsqueeze(0),
                                      zbt[32 * j : 32 * j + 1, :])

        # final d (t = T-1) lives in dtail column 0
        for j in range(4):
            nc.sync.dma_start(d_out[T // SB, j, 0:N].unsqueeze(0),
                              dtail[32 * j : 32 * j + 1, 0:N])

    return d_out


def gather_out(scratch, cfg):
    """d_scratch [T/SB+1, 4, SB*N] (tanh form, shifted by one) -> [B, T]."""
    T, SB, N = cfg.T, cfg.sblk, cfg.ncol
    sc = np.asarray(scratch).reshape(T // SB + 1, 4, SB, N)
    seq = sc.transpose(1, 3, 0, 2).reshape(4, N, (T // SB + 1) * SB)
    vals = seq[:, :, 1 : T + 1]                   # drop the t=-1 init slot
    return (0.5 + 0.5 * vals).reshape(cfg.B, T).astype(np.float32)


# ----------------------------------------------------------------------------
# numpy reference of the exact kernel math (for mini-tests)
# ----------------------------------------------------------------------------
def numpy_model(x, params):
    """x [B, T, 4] -> [B, T] float32, same math as reference()."""
    import jax
    import jax.numpy as jnp
    B, T, F = x.shape
    H = params["W_hh"].shape[1]
    inv_std = 1.0 / np.sqrt(params["bn_var"] + EPS)

    h = np.zeros((B, H), np.float32)
    c = np.zeros((B, H), np.float32)
    d = np.zeros((B, 1), np.float32)
    outs = np.zeros((B, T), np.float32)
    sig = lambda v: 1.0 / (1.0 + np.exp(-v))
    for t in range(T):
        z = np.concatenate([x[:, t], d], 1)
        z = (z - params["bn_mean"]) * inv_std * params["bn_gamma"] + params["bn_beta"]
        gates = z @ params["W_ih"].T + params["b_ih"] + h @ params["W_hh"].T + params["b_hh"]
        i, f, g, o = np.split(gates, 4, 1)
        c = sig(f) * c + sig(i) * np.tanh(g)
        h = sig(o) * np.tanh(c)
        d = sig(np.maximum(h @ params["W1"].T + params["b1"], 0) @ params["W2"].T + params["b2"])
        outs[:, t] = d[:, 0]
    return outs


# ----------------------------------------------------------------------------
# Entry point
# ----------------------------------------------------------------------------
_CACHE = {}


def _get_compiled():
    if "nc" not in _CACHE:
        nc = bacc.Bacc("TRN2", target_bir_lowering=False, debug=False)
        build_kernel(nc, FULL)
        nc.compile()
        _CACHE["nc"] = nc
    return _CACHE["nc"]


def kernel(**inputs):
    from concourse.bass_utils import run_bass_kernel_spmd

    x = np.asarray(inputs["x"], np.float32)
    B, T, F = x.shape
    ncores = 8
    bc = B // ncores
    folded = fold_params(inputs)
    b2h = np.full((128, 1), folded["b2half"], np.float32)

    nc = _get_compiled()
    in_maps = []
    for c in range(ncores):
        m = {
            "xprep": prep_x(x[c * bc : (c + 1) * bc], FULL),
            "whh": folded["whh"],
            "wx1d": folded["wx1d"],
            "wm1": folded["wm1"],
            "wm2": folded["wm2"],
            "b1col": folded["b1col"],
            "b2half": b2h,
        }
        in_maps.append(m)

    res = run_bass_kernel_spmd(nc, in_maps, list(range(ncores)))
    outs = [gather_out(res.results[c]["d_scratch"], FULL) for c in range(ncores)]
    return np.concatenate(outs, 0)[:, :, None].astype(np.float32)
